# revision 60
# baseline (speedup 1.0000x reference)
"""Trainium2 Bass kernel for nn_GRUWithAttentionModel (B=4,D=60,S=512,F=158,C=64,H=128).

Sharding: phase 1 (per-day attention -> market vector) is sharded over the 240
(day,batch) pairs, 30 per core; the tiny (64,240) market matrix is AllGathered;
phase 2/3 (layernorm + GRU over days + head) is sharded over the 512 stocks,
64 per core (256 sequences/core).

All bulk tensors are staged bf16 (inputs ~N(0,1), tolerance 2e-2); matmul
accumulation stays fp32 in PSUM. x is preloaded into SBUF with 4 large DMAs;
the GRU day loop runs zero DMAs.

Self-contained: call kernel(**inputs) with the full reference.setup_inputs()
arrays; returns the full (4,512,1) output.
"""
import numpy as np
import ml_dtypes

from concourse import bacc, tile, mybir
from concourse.bass import ts
from concourse.bass_utils import run_bass_kernel_spmd

F32 = mybir.dt.float32
F32R = mybir.dt.float32r
BF16 = mybir.dt.bfloat16
BF = ml_dtypes.bfloat16

B, D, S, F, C, H, OUT = 4, 60, 512, 158, 64, 128, 1
LN_EPS = 1e-5
NCORE = 8
PPC = (B * D) // NCORE      # 30 pairs/core, pair p = d*4+b (d-major)
SL = S // NCORE             # 64 stocks/core
N = B * SL                  # 256 sequences/core
FC = F + C                  # 222
G3 = 3 * H                  # 384
F1 = F - 128                # 30 leftover feature rows
FA1 = F1 + 1                # +1 ones row (carries QKV biases)


def _r(ap):
    return ap.bitcast(F32R)


# ---------------------------------------------------------------- host prep
def host_prep(x, feature_mask, wq, bq, wk, bk, wv, bv, ln_g, ln_b,
              w_ih, w_hh, b_ih, b_hh, w1, b1, w2, b2):
    f32 = np.float32
    x = np.asarray(x, f32)
    x_att = x.transpose(1, 0, 2, 3).reshape(B * D, S, F)          # p = d*4+b
    mask_p = np.asarray(feature_mask, f32).transpose(1, 0, 2).reshape(B * D, S)
    denom = np.maximum(mask_p.sum(1), 1.0).astype(f32)
    u = (mask_p / denom[:, None]).astype(f32)
    mneg = ((1.0 - mask_p) * np.float32(-2e9)).astype(f32)

    wp = (np.asarray(w_ih, f32) * np.asarray(ln_g, f32)[None, :])
    A = wp.sum(1).astype(f32)
    Bb = (np.asarray(w_ih, f32) @ np.asarray(ln_b, f32) + np.asarray(b_ih, f32))
    Bb = Bb.copy()
    Bb[:2 * H] += np.asarray(b_hh, f32)[:2 * H]
    # LT rows: x-feats 0:128 | [market 64, x-feats 128:158 (30), pad2, Bb@96]
    LT0 = np.ascontiguousarray(wp.T[:128]).astype(BF)             # (128,384)
    LT1 = np.ascontiguousarray(np.concatenate(
        [wp.T[F:FC], wp.T[128:F], np.zeros((2, G3), f32), Bb[None]],
        0)).astype(BF)                                            # (97,384)
    AT = np.ascontiguousarray(A[None]).astype(BF)                 # (1,384)

    bqkv = np.stack([np.asarray(bq, f32), np.asarray(bk, f32),
                     np.asarray(bv, f32)], 0).reshape(1, 192)    # [bq|bk|bv]

    per_core = []
    for i in range(NCORE):
        pi = slice(PPC * i, PPC * (i + 1))
        sl = slice(SL * i, SL * (i + 1))
        xs_sl = x[:, :, sl, :]
        xsum = (xs_sl.sum(-1) / np.float32(FC)).astype(f32)
        xsq = ((xs_sl * xs_sl).sum(-1) / np.float32(FC)).astype(f32)
        # (158, PPC*512): col = p_local*512 + stock
        xatt_T = np.ascontiguousarray(
            x_att[pi].transpose(2, 0, 1).reshape(F, PPC * S)).astype(BF)
        # (158, D*N): col = d*256 + seq
        xseq_T = np.ascontiguousarray(
            xs_sl.transpose(3, 1, 0, 2).reshape(F, D * N)).astype(BF)
        # rows 0:31 = [x-att 128:158, ones]; rows 64:94 = x-seq 128:158
        xmix = np.concatenate(
            [xatt_T[128:F], np.ones((1, PPC * S), BF),
             np.zeros((33, PPC * S), BF), xseq_T[128:F]], 0)      # (94, .)
        per_core.append(dict(
            xa0=xatt_T[0:128], xmix=np.ascontiguousarray(xmix),
            xd0=xseq_T[0:128],
            uT=np.ascontiguousarray(
                u[pi].reshape(PPC, 4, 128).transpose(2, 0, 1).reshape(128, PPC * 4)),
            mnegT=np.ascontiguousarray(
                mneg[pi].reshape(PPC, 4, 128).transpose(2, 0, 1).reshape(128, PPC * 4)),
            xs=np.ascontiguousarray(xsum.transpose(1, 0, 2).reshape(D, N)),
            xq=np.ascontiguousarray(xsq.transpose(1, 0, 2).reshape(D, N)),
            LT0=LT0, LT1=LT1, AT=AT,
            WHH=np.ascontiguousarray(np.asarray(w_hh, f32).T).astype(BF),
            bhh_n=np.ascontiguousarray(np.asarray(b_hh, f32)[2 * H:][:, None]),
            WQKV0=np.ascontiguousarray(np.concatenate(
                [wq[:128], wk[:128], wv[:128]], 1).astype(f32)).astype(BF),
            WQKV1=np.ascontiguousarray(np.concatenate(
                [np.concatenate([wq[128:], wk[128:], wv[128:]], 1), bqkv],
                0).astype(f32)).astype(BF),                       # (31,192)
            W1=np.ascontiguousarray(np.asarray(w1, f32)).astype(BF),
            B1=np.ascontiguousarray(np.asarray(b1, f32)[:, None]),
            W2=np.ascontiguousarray(np.asarray(w2, f32)).astype(BF),
            c222v=np.full((C, 1), 1.0 / FC, f32),
            B2=np.ascontiguousarray(np.asarray(b2, f32)[None, :]),
            ident=np.eye(128, dtype=f32),
            identb=np.eye(128, dtype=f32).astype(BF),
        ))
    return per_core


INPUT_SPECS = dict(
    xa0=((128, PPC * S), BF16), xmix=((94, PPC * S), BF16),
    xd0=((128, D * N), BF16),
    uT=((128, PPC * 4), F32), mnegT=((128, PPC * 4), F32),
    xs=((D, N), F32), xq=((D, N), F32),
    LT0=((128, G3), BF16), LT1=((97, G3), BF16), AT=((1, G3), BF16),
    WHH=((H, G3), BF16), bhh_n=((H, 1), F32),
    WQKV0=((128, 192), BF16), WQKV1=((FA1, 192), BF16),
    W1=((H, C), BF16), B1=((C, 1), F32), W2=((C, 1), BF16), B2=((1, 1), F32),
    c222v=((C, 1), F32), ident=((128, 128), F32), identb=((128, 128), BF16),
)


# ---------------------------------------------------------------- program
def build_program():
    nc = bacc.Bacc("TRN2", target_bir_lowering=False, debug=False,
                   num_devices=NCORE)
    dram = {k: nc.dram_tensor(k, list(shp), dt, kind="ExternalInput").ap()
            for k, (shp, dt) in INPUT_SPECS.items()}
    yout = nc.dram_tensor("yout", [1, N], F32, kind="ExternalOutput").ap()
    AL = mybir.AluOpType
    AF = mybir.ActivationFunctionType

    with tile.TileContext(nc) as tc:
        with (
            nc.allow_low_precision(reason="bf16 staging within 2e-2 tolerance"),
            tc.tile_pool(name="const", bufs=1) as cp,
            tc.tile_pool(name="dram", bufs=1, space="DRAM") as dp,
        ):
            # ---- persistent tiles (weights + preloaded activations)
            # load order = need order: QKV weights + first x chunks first,
            # phase-2-only tensors last (overlap with phase-1 compute)
            cst = {}
            for k in INPUT_SPECS:
                shp, dt = INPUT_SPECS[k]
                cst[k] = cp.tile(list(shp), dt, tag=k, name=k)
            for k in ("WQKV0", "WQKV1", "mnegT", "ident"):
                nc.sync.dma_start(cst[k][:], dram[k])
            NCH = 10
            for ci in range(NCH):
                for k in ("xa0", "xmix"):
                    cw = INPUT_SPECS[k][0][1] // NCH
                    nc.sync.dma_start(cst[k][:, cw * ci:cw * (ci + 1)],
                                      dram[k][:, cw * ci:cw * (ci + 1)])
                if ci == 0:
                    nc.sync.dma_start(cst["uT"][:], dram["uT"])
            for k in ("xd0", "xs", "xq", "LT0", "LT1", "AT", "WHH", "bhh_n",
                      "W1", "B1", "W2", "B2", "identb"):
                nc.sync.dma_start(cst[k][:], dram[k])
            nc.sync.dma_start(cst["c222v"][:], dram["c222v"])
            onesb = cp.tile([128, 1], F32, tag="onesb")
            nc.vector.memset(onesb[:], 1.0)
            epsc = cp.tile([D, 1], F32, tag="epsc")
            nc.vector.memset(epsc[:], LN_EPS)
            mcols = cp.tile([C, PPC], F32, tag="mcols")
            market = cp.tile([C, B * D], F32, tag="market")
            market_bf = cp.tile([C, B * D], BF16, tag="market_bf")
            summc = cp.tile([D, 4], F32, tag="summc")
            sumsqc = cp.tile([D, 4], F32, tag="sumsqc")
            rsr2 = cp.tile([D, 2 * N], BF16, tag="rsr2")  # [rstd | row2]
            # au1 ping-pong: [market*psr (64); x*psr (30); pad2; ones @96]
            au1s = []
            for k in range(2):
                t = cp.tile([97, N], BF16, tag=f"au1_{k}")
                nc.vector.memset(t[:], 0.0)
                nc.vector.memset(t[96:97, :], 1.0)
                au1s.append(t)
            rsrow = cp.tile([1, D * 2 * N], BF16, tag="rsrow")

            # ================= phase 1: attention -> market columns
            # two-stage software pipeline across pairs:
            #   stage A(p): QKV matmuls + Q/K/V^T evacuations
            #   stage B(p): scores/exp/ctx + market tail
            with (
                tc.tile_pool(name="w1p", bufs=2) as wp,
                tc.tile_pool(name="ps1", bufs=1, space="PSUM") as ps,
            ):
                def stageA(p):
                    xa0 = cst["xa0"][:, ts(p, S)]
                    xa1 = cst["xmix"][0:FA1, ts(p, S)]
                    pq = ps.tile([C, S], F32, tag="qk0", name="pq")
                    pk = ps.tile([C, S], F32, tag="qk1", name="pk")
                    for j, pqk in enumerate((pq, pk)):
                        nc.tensor.matmul(pqk[:], cst["WQKV0"][:, ts(j, C)],
                                         xa0, start=True, stop=False)
                        nc.tensor.matmul(pqk[:], cst["WQKV1"][:, ts(j, C)],
                                         xa1, start=False, stop=True)
                    q_sb = wp.tile([C, S], BF16, tag="q", name="q_sb")
                    nc.scalar.copy(q_sb[:], pq[:])
                    k_sb = wp.tile([C, S], BF16, tag="k", name="k_sb")
                    nc.vector.tensor_copy(k_sb[:], pk[:])
                    v2e = []
                    for half in range(2):
                        pv2 = ps.tile([128, 128], F32, tag="tp0", name="pv2")
                        for j in range(2):
                            c = 2 * half + j
                            nc.tensor.matmul(pv2[:, ts(j, C)],
                                             xa0[:, ts(c, 128)],
                                             cst["WQKV0"][:, ts(2, C)],
                                             start=True, stop=False)
                            nc.tensor.matmul(pv2[:, ts(j, C)],
                                             xa1[:, ts(c, 128)],
                                             cst["WQKV1"][:, ts(2, C)],
                                             start=False, stop=True)
                        ve = wp.tile([128, 2 * (C + 1)], BF16,
                                     tag=f"v2e{half}", name="ve")
                        nc.vector.tensor_copy(
                            ve[:].rearrange("p (g c) -> p g c", g=2)[:, :, 0:C],
                            pv2[:].rearrange("p (g c) -> p g c", g=2))
                        nc.gpsimd.memset(
                            ve[:].rearrange("p (g c) -> p g c", g=2)
                            [:, :, C:C + 1], 1.0)
                        v2e.append(ve)
                    return q_sb, k_sb, v2e

                def stageB(p, st):
                    q_sb, k_sb, v2e = st
                    pss, eT = [], []
                    for c in range(4):
                        pt = ps.tile([128, S], F32, tag=f"ss{c % 2}",
                                     name="pss")
                        nc.tensor.matmul(pt[:], k_sb[:, ts(c, 128)],
                                         q_sb[:], start=True, stop=True)
                        pss.append(pt)
                    for c in range(4):
                        et = wp.tile([128, S], BF16, tag=f"eT{c}", name="et")
                        nc.scalar.activation(
                            et[:], pss[c][:], AF.Exp, scale=0.125,
                            bias=cst["mnegT"][:, 4 * p + c:4 * p + c + 1])
                        eT.append(et)
                    psc = ps.tile([C + 1, S], F32, tag="ctx", name="psc")
                    for c in range(4):
                        nc.tensor.matmul(psc[:],
                                         v2e[c // 2][:, ts(c % 2, C + 1)],
                                         eT[c][:],
                                         start=(c == 0), stop=(c == 3))
                    ctxr = wp.tile([C + 1, S], F32, tag="ctxr", name="ctxr")
                    nc.vector.tensor_copy(ctxr[:], psc[:])
                    prt = ps.tile([128, 4], F32, tag="ctx", name="prt")
                    for c in range(4):
                        nc.tensor.matmul(prt[:, c:c + 1],
                                         ctxr[C:C + 1, ts(c, 128)],
                                         onesb[C:C + 1, 0:1],
                                         start=True, stop=True)
                    rr = wp.tile([128, 4], F32, tag="rr", name="rr")
                    nc.vector.reciprocal(rr[:], prt[:])
                    gT = wp.tile([128, 4], BF16, tag="gT", name="gT")
                    nc.vector.tensor_tensor(out=gT[:],
                                            in0=cst["uT"][:, 4 * p:4 * p + 4],
                                            in1=rr[:], op=AL.mult)
                    psm = ps.tile([C, 1], F32, tag="mkt", name="psm")
                    cxs = []
                    for half in range(2):
                        pct = ps.tile([128, 128], F32, tag="tp2", name="pct")
                        for j in range(2):
                            c = 2 * half + j
                            nc.tensor.transpose(pct[:, ts(j, C)],
                                                ctxr[0:C, ts(c, 128)],
                                                cst["ident"][0:C, 0:C])
                        cx = wp.tile([128, 128], BF16, tag=f"cx{half}",
                                     name="cx")
                        nc.vector.tensor_copy(cx[:], pct[:])
                        cxs.append(cx)
                    for c in range(4):
                        nc.tensor.matmul(psm[:], cxs[c // 2][:, ts(c % 2, C)],
                                         gT[:, c:c + 1],
                                         start=(c == 0), stop=(c == 3))
                    nc.vector.tensor_copy(mcols[:, p:p + 1], psm[:])

                st = stageA(0)
                for p in range(PPC):
                    nxt = stageA(p + 1) if p + 1 < PPC else None
                    stageB(p, st)
                    st = nxt

            # ================= collective: market_cols -> full market
            cin = dp.tile([C, PPC], F32)
            call = dp.tile([NCORE * C, PPC], F32)
            nc.sync.dma_start(cin[:], mcols[:])
            nc.gpsimd.collective_compute(
                "AllGather", mybir.AluOpType.bypass,
                replica_groups=[list(range(NCORE))],
                ins=[cin[:].opt()], outs=[call[:].opt()])
            call_v = call[:].rearrange("(blk c) j -> blk c j", blk=NCORE)
            nc.sync.dma_start(
                market[:].rearrange("c (blk j) -> c blk j", blk=NCORE),
                call_v.transpose([1, 0, 2]))

            # ================= phase 2/3: LN stats + GRU + head
            with (
                tc.tile_pool(name="w3p", bufs=2) as w3,
                tc.tile_pool(name="ps3", bufs=2, space="PSUM") as ps,
            ):
                nc.vector.tensor_copy(market_bf[:], market[:])
                # market sums (scaled by 1/222) -> (60,4) layout
                msq = w3.tile([C, B * D], F32, tag="msq")
                nc.scalar.square(msq[:], market[:])
                for src, dst in ((market, summc), (msq, sumsqc)):
                    psum = ps.tile([1, B * D], F32, tag="R0", bufs=1)
                    nc.tensor.matmul(psum[:], cst["c222v"][:], src[:],
                                     start=True, stop=True)
                    srow = w3.tile([1, B * D], F32, tag="srow")
                    nc.vector.tensor_copy(srow[:], psum[:])
                    # (1,240) row -> (60,4): per-b strided transpose matmuls
                    pmin = ps.tile([D, 4], F32, tag="Z0", bufs=1)
                    srow_v = srow[:].rearrange("o (d b) -> o d b", b=4)
                    for b in range(4):
                        nc.tensor.matmul(pmin[:, b:b + 1],
                                         srow_v[0:1, :, b],
                                         onesb[0:1, 0:1],
                                         start=True, stop=True)
                    nc.vector.tensor_copy(dst[:], pmin[:])

                # LN statistics on (60,256) tiles
                mu = w3.tile([D, N], F32, tag="mu")
                nc.vector.tensor_tensor(
                    out=mu[:].rearrange("p (b s) -> p b s", b=4),
                    in0=cst["xs"][:].rearrange("p (b s) -> p b s", b=4),
                    in1=summc[:].unsqueeze(2).broadcast_to([D, 4, SL]),
                    op=AL.add)
                ms = w3.tile([D, N], F32, tag="ms")
                nc.vector.tensor_tensor(
                    out=ms[:].rearrange("p (b s) -> p b s", b=4),
                    in0=cst["xq"][:].rearrange("p (b s) -> p b s", b=4),
                    in1=sumsqc[:].unsqueeze(2).broadcast_to([D, 4, SL]),
                    op=AL.add)
                mu2 = w3.tile([D, N], F32, tag="mu2")
                nc.vector.tensor_tensor(out=mu2[:], in0=mu[:], in1=mu[:],
                                        op=AL.mult)
                var = w3.tile([D, N], F32, tag="var")
                nc.vector.tensor_tensor(out=var[:], in0=ms[:], in1=mu2[:],
                                        op=AL.subtract)
                std = w3.tile([D, N], F32, tag="std")
                nc.scalar.activation(std[:], var[:], AF.Sqrt, bias=epsc[:])
                rstd = w3.tile([D, N], F32, tag="rstd")
                nc.vector.reciprocal(rstd[:], std[:])
                nc.vector.tensor_copy(rsr2[:, 0:N], rstd[:])
                nc.vector.scalar_tensor_tensor(
                    out=rsr2[:, N:2 * N], in0=rstd[:], scalar=-1.0, in1=mu[:],
                    op0=AL.mult, op1=AL.mult)
                rs_dram = dp.tile([D, 2 * N], BF16, name="rs_dram")
                nc.sync.dma_start(rs_dram[:], rsr2[:])
                nc.sync.dma_start(
                    rsrow[:], rs_dram[:].rearrange("p f -> () (p f)"))

                # ---- GRU over days (x-side prepped one day ahead)
                h = [None, None]
                for k in range(2):
                    h[k] = w3.tile([H, N // 2], BF16, tag=f"h{k}")
                    nc.vector.memset(h[k][:], 0.0)

                def prep(d):
                    au1 = au1s[d % 2]
                    psr = w3.tile([128, N], BF16, tag="psr")
                    src = (rsr2[0:1, 0:N] if d == 0 else
                           rsrow[0:1, 2 * N * d:2 * N * d + N])
                    nc.gpsimd.partition_broadcast(psr[:], src)
                    au0 = w3.tile([128, N], BF16, tag="au0")
                    nc.vector.tensor_tensor(out=au0[:],
                                            in0=cst["xd0"][:, ts(d, N)],
                                            in1=psr[:], op=AL.mult)
                    nc.vector.tensor_tensor(
                        out=au1[0:C, :].rearrange("p (b s) -> p b s", b=4),
                        in0=market_bf[:, 4 * d:4 * d + 4].unsqueeze(2)
                            .broadcast_to([C, 4, SL]),
                        in1=psr[0:C, :].rearrange("p (b s) -> p b s", b=4),
                        op=AL.mult)
                    nc.vector.tensor_tensor(out=au1[C:C + F1, :],
                                            in0=cst["xmix"][64:94, ts(d, N)],
                                            in1=psr[C:C + F1, :], op=AL.mult)
                    return au0, au1

                HF = N // 2  # column half: two independent chains
                pre = prep(0)
                for d in range(D):
                    au0, au1 = pre
                    R, Z, XN, HN = ([None, None] for _ in range(4))
                    for k in range(2):
                        R[k] = ps.tile([128, HF], F32, tag=f"R{k}",
                                       bufs=1, name=f"R{k}")
                        Z[k] = ps.tile([128, HF], F32, tag=f"Z{k}",
                                       bufs=1, name=f"Z{k}")
                        XN[k] = ps.tile([128, HF], F32, tag=f"XN{k}",
                                        bufs=1, name=f"XN{k}")
                        HN[k] = ps.tile([128, HF], F32, tag=f"HN{k}",
                                        bufs=1, name=f"HN{k}")
                    r2src = rsr2 if d == 0 else rsrow
                    r2off = N if d == 0 else 2 * N * d + N
                    row2a = r2src[0:1, r2off:r2off + HF]
                    row2b = r2src[0:1, r2off + HF:r2off + 2 * HF]
                    for gc, P in enumerate((R, Z, XN)):
                        for k in range(2):
                            cc = ts(k, HF)
                            nc.tensor.matmul(P[k][:], cst["LT0"][:, ts(gc, 128)],
                                             au0[:, cc], start=True, stop=False)
                            nc.tensor.matmul(P[k][:], cst["LT1"][:, ts(gc, 128)],
                                             au1[:, cc], start=False, stop=False)
                            nc.tensor.matmul(P[k][:], cst["AT"][0:1, ts(gc, 128)],
                                             (row2a, row2b)[k],
                                             start=False, stop=False)
                    for k in range(2):
                        nc.tensor.matmul(R[k][:], cst["WHH"][:, 0:128],
                                         h[k][:], start=False, stop=True)
                        nc.tensor.matmul(Z[k][:], cst["WHH"][:, 128:256],
                                         h[k][:], start=False, stop=True)
                        nc.tensor.matmul(HN[k][:], cst["WHH"][:, 256:384],
                                         h[k][:], start=True, stop=True)
                    if d + 1 < D:
                        pre = prep(d + 1)

                    r_sb, z_sb, zc, zh, t1, n_sb, t3, h_new = (
                        [None, None] for _ in range(8))
                    for k in range(2):
                        r_sb[k] = w3.tile([H, HF], F32, tag=f"r{k}")
                        nc.scalar.activation(r_sb[k][:], R[k][:], AF.Sigmoid)
                    for k in range(2):
                        z_sb[k] = w3.tile([H, HF], BF16, tag=f"z{k}")
                        nc.scalar.activation(z_sb[k][:], Z[k][:], AF.Sigmoid)
                    for k in range(2):
                        t1[k] = w3.tile([H, HF], BF16, tag=f"t1{k}")
                        nc.vector.scalar_tensor_tensor(
                            out=t1[k][:], in0=HN[k][:],
                            scalar=cst["bhh_n"][:], in1=r_sb[k][:],
                            op0=AL.add, op1=AL.mult)
                        nc.tensor.matmul(XN[k][:], cst["identb"][:],
                                         t1[k][:], start=False, stop=True)
                        zc[k] = w3.tile([H, HF], BF16, tag=f"zc{k}")
                        nc.vector.tensor_scalar(out=zc[k][:], in0=z_sb[k][:],
                                                scalar1=-1.0, scalar2=1.0,
                                                op0=AL.mult, op1=AL.add)
                        zh[k] = w3.tile([H, HF], BF16, tag=f"zh{k}")
                        nc.gpsimd.tensor_tensor(out=zh[k][:], in0=z_sb[k][:],
                                                in1=h[k][:], op=AL.mult)
                    for k in range(2):
                        n_sb[k] = w3.tile([H, HF], BF16, tag=f"n{k}")
                        nc.scalar.activation(n_sb[k][:], XN[k][:], AF.Tanh)
                        t3[k] = w3.tile([H, HF], BF16, tag=f"t3{k}")
                        nc.vector.tensor_tensor(out=t3[k][:], in0=n_sb[k][:],
                                                in1=zc[k][:], op=AL.mult)
                        h_new[k] = w3.tile([H, HF], BF16, tag=f"h{k}")
                        nc.vector.tensor_tensor(out=h_new[k][:], in0=t3[k][:],
                                                in1=zh[k][:], op=AL.add)
                    h = h_new

                # ---- head
                phd = ps.tile([C, N], F32, tag="XN0", bufs=1)
                for k in range(2):
                    nc.tensor.matmul(phd[:, ts(k, HF)], cst["W1"][:],
                                     h[k][:], start=True, stop=True)
                hid = w3.tile([C, N], BF16, tag="hid")
                nc.scalar.activation(hid[:], phd[:], AF.Relu,
                                     bias=cst["B1"][:])
                pso = ps.tile([1, N], F32, tag="R0", bufs=1)
                nc.tensor.matmul(pso[:], cst["W2"][:], hid[:],
                                 start=True, stop=True)
                yo = w3.tile([1, N], F32, tag="yo")
                nc.scalar.activation(yo[:], pso[:], AF.Identity,
                                     bias=cst["B2"][0:1, 0:1])
                nc.sync.dma_start(yout, yo[:])

    nc.compile()
    return nc


_NC_CACHE = None


def kernel(**inputs):
    global _NC_CACHE
    per_core = host_prep(**inputs)
    if _NC_CACHE is None:
        _NC_CACHE = build_program()
    nc = _NC_CACHE
    in_maps = [{k: pc[k] for k in INPUT_SPECS} for pc in per_core]
    res = run_bass_kernel_spmd(nc, in_maps, list(range(NCORE)))
    out = np.zeros((B, S, OUT), np.float32)
    for i in range(NCORE):
        out[:, SL * i:SL * (i + 1), 0] = res.results[i]["yout"].reshape(B, SL)
    return out
